# revision 1
# baseline (speedup 1.0000x reference)
"""Trainium2 Bass kernel for a two-branch cross-attention block.

Math (per branch pair):
    x1n = LN(x1); x2n = LN(x2)
    q1,k1,v1 = split(x1n @ w_qkv1); q2,k2,v2 = split(x2n @ w_qkv2)
    out1 = softmax(q1 k2^T * s) v2 @ w_out1 + b_out1
    out2 = softmax(q2 k1^T * s) v1 @ w_out2 + b_out2

Sharding: 8 cores = 4 batches x 2 head-groups (8 heads each). Each core
handles both branches for its (batch, head-group); the out-projection
contracts over heads, so each core produces a partial [2048, 1024] per
branch and the host sums the two head-group partials + bias.

LN affine (g, b) is folded into the QKV weights on the host
(W' = diag(g) W, bias' = b W), so the device only computes the pure
normalization z = (x - mu) * rstd. The softmax scale is folded into the
q-side weights. QKV biases enter via K=1 ones-row matmuls.

The two output branches run sequentially on-device, recomputing LN per
use, so the persistent q/k/v/attn tiles fit SBUF (tag-shared slots).

On-device dataflow per output branch (matmul inputs bf16, fp32 accum):
    LN (DVE/ACT) -> PE transpose -> xnT [feat, tok]
    qT, kT via W as stationary; v natural via xnT as stationary
    per head: S^T[j,i] = kT_h^T q_h (K=64) -> exp (ACT, PSUM->SBUF bf16)
              AV: out[65, i] += v_aug[j,:65]^T @ expS^T (col 64 = ones -> Z)
              recipZ = exp(-ln(Z)); DMA partition-broadcast; scale on DVE
    out-proj: attn_T as stationary, accumulate 4 hd-chunks.
"""

import sys
from contextlib import ExitStack

import numpy as np
import ml_dtypes

sys.path.insert(0, "/opt/trn_rl_repo")
sys.path.insert(0, "/opt/trn_rl_repo/concourse")

import concourse.bass as bass
import concourse.tile as tile
from concourse import bacc, mybir
from concourse.bass import ds, ts
from concourse.masks import make_identity

F32 = mybir.dt.float32
BF16 = mybir.dt.bfloat16
AF = mybir.ActivationFunctionType
ALU = mybir.AluOpType

B, N, DIM = 4, 2048, 1024
HEADS, DH = 16, 64
SCALE = DH ** -0.5
HPC = 8          # heads per core
QKCOLS = HPC * DH  # 512 qkv columns per core per tensor
TC = N // 128    # 16 token chunks
KC = DIM // 128  # 8 feature chunks
EPS = 1e-5


def build_program():
    nc = bacc.Bacc(
        "TRN2",
        target_bir_lowering=False,
        debug=False,
        enable_asserts=True,
        num_devices=8,
    )
    xs, wq, wk, wv, bq, bk, bv, wo, outs = [], [], [], [], [], [], [], [], []
    for br in range(2):
        xs.append(nc.dram_tensor(f"x{br}", [N, DIM], F32, kind="ExternalInput").ap())
        wq.append(nc.dram_tensor(f"wq{br}", [DIM, QKCOLS], BF16, kind="ExternalInput").ap())
        wk.append(nc.dram_tensor(f"wk{br}", [DIM, QKCOLS], BF16, kind="ExternalInput").ap())
        wv.append(nc.dram_tensor(f"wv{br}", [DIM, QKCOLS], BF16, kind="ExternalInput").ap())
        bq.append(nc.dram_tensor(f"bq{br}", [1, QKCOLS], BF16, kind="ExternalInput").ap())
        bk.append(nc.dram_tensor(f"bk{br}", [1, QKCOLS], BF16, kind="ExternalInput").ap())
        bv.append(nc.dram_tensor(f"bv{br}", [1, QKCOLS], BF16, kind="ExternalInput").ap())
        wo.append(nc.dram_tensor(f"wo{br}", [QKCOLS, DIM], BF16, kind="ExternalInput").ap())
        outs.append(nc.dram_tensor(f"o{br}", [N, DIM], F32, kind="ExternalOutput").ap())
    # DRAM staging for the per-head recipZ rows (DMA partition-broadcast
    # needs a DRAM source); one row per (ob, h) so there is no reuse.
    zst = nc.dram_tensor("zstage", [2 * HPC, N], F32, kind="Internal").ap()

    with tile.TileContext(nc) as tc:
        with ExitStack() as ctx:
            _body(ctx, tc, xs, wq, wk, wv, bq, bk, bv, wo, outs, zst)
    nc.finalize()
    return nc


def _body(ctx, tc, xs, wq, wk, wv, bq, bk, bv, wo, outs, zst):
    nc = tc.nc
    p_const = ctx.enter_context(tc.tile_pool(name="const", bufs=1))
    p_x = ctx.enter_context(tc.tile_pool(name="x", bufs=3))
    p_stat = ctx.enter_context(tc.tile_pool(name="stat", bufs=4))
    p_z = ctx.enter_context(tc.tile_pool(name="z", bufs=3))
    p_xnT = ctx.enter_context(tc.tile_pool(name="xnT", bufs=1))
    p_w = ctx.enter_context(tc.tile_pool(name="w", bufs=1))
    p_big = ctx.enter_context(tc.tile_pool(name="big", bufs=1))
    p_es = ctx.enter_context(tc.tile_pool(name="es", bufs=3))
    p_rz = ctx.enter_context(tc.tile_pool(name="rz", bufs=1))
    p_outst = ctx.enter_context(tc.tile_pool(name="outst", bufs=2))
    ps_mm = ctx.enter_context(tc.tile_pool(name="ps_mm", bufs=2, space="PSUM"))
    ps_av = ctx.enter_context(tc.tile_pool(name="ps_av", bufs=1, space="PSUM"))

    ident = p_const.tile([128, 128], BF16, tag="ident", name="ident")
    make_identity(nc, ident)
    ones = p_const.tile([1, 512], BF16, tag="ones", name="ones")
    nc.vector.memset(ones, 1.0)
    epst = p_const.tile([128, 1], F32, tag="eps", name="epst")
    nc.vector.memset(epst, EPS)

    def phase_A(br, seg):
        """LN + transpose -> xnT [128, kc, tokens] (bf16).

        Two passes over x (re-DMA'd) so the 16 per-tile Ln/Exp rstd calls
        batch into ONE Ln + ONE Exp -- ACT table sets reload on every
        Ln<->Exp alternation (~2.7us each), which dominated ScalarE time.
        """
        xnT = p_xnT.tile([128, KC, N], BF16, tag="xnT", name=f"xnT_{seg}")
        stats = p_stat.tile([128, TC, 2], F32, tag="stats", name=f"stats_{seg}")
        rstd = p_stat.tile([128, TC], F32, tag="rstd", name=f"rstd_{seg}")
        for t in range(TC):
            xt = p_x.tile([128, DIM], F32, tag="xt", name=f"xt{seg}_{t}")
            nc.sync.dma_start(out=xt, in_=xs[br][ts(t, 128), :])
            st = p_stat.tile([128, 2, 6], F32, tag="st", name=f"st{seg}_{t}")
            for sg in range(2):
                nc.vector.bn_stats(out=st[:, sg, :], in_=xt[:, ts(sg, 512)])
            nc.vector.bn_aggr(out=stats[:, t, :], in_=st)
        # rstd = exp(-0.5 * ln(var + eps)), batched over all 16 tiles
        nc.scalar.activation(out=rstd, in_=stats[:, :, 1], func=AF.Ln,
                             bias=epst, scale=1.0)
        nc.scalar.activation(out=rstd, in_=rstd, func=AF.Exp, scale=-0.5)
        for t in range(TC):
            xt = p_x.tile([128, DIM], F32, tag="xt", name=f"xt2{seg}_{t}")
            nc.sync.dma_start(out=xt, in_=xs[br][ts(t, 128), :])
            zt = p_z.tile([128, DIM], BF16, tag="zt", name=f"zt{seg}_{t}")
            nc.vector.tensor_scalar(out=zt, in0=xt, scalar1=stats[:, t, 0:1],
                                    scalar2=rstd[:, t:t + 1],
                                    op0=ALU.subtract, op1=ALU.mult)
            ptr = ps_mm.tile([128, KC, 128], BF16, tag="mm", name=f"ptr{seg}_{t}")
            for fc in range(KC):
                nc.tensor.transpose(out=ptr[:, fc, :], in_=zt[:, ts(fc, 128)],
                                    identity=ident)
            nc.vector.tensor_copy(out=xnT[:, :, ts(t, 128)], in_=ptr)
        return xnT

    def phase_B(xnT, wt_d, bias_d, dstT, lbl):
        """q or k projection, transposed output layout."""
        w_re = wt_d.rearrange("(kc p) c -> p kc c", p=128)
        for cc in range(4):
            wsb = p_w.tile([128, KC, 128], BF16, tag="w", bufs=2,
                           name=f"w_{lbl}_{cc}")
            nc.sync.dma_start(out=wsb, in_=w_re[:, :, ts(cc, 128)])
            bsb = p_w.tile([1, 128], BF16, tag="b", bufs=2, name=f"b_{lbl}_{cc}")
            nc.sync.dma_start(out=bsb, in_=bias_d[:, ts(cc, 128)])
            for ih in range(2):
                ps = ps_mm.tile([128, 1024], F32, tag="mm",
                                name=f"psB_{lbl}_{cc}_{ih}")
                for i2 in range(2):
                    nc.tensor.matmul(out=ps[:, ts(i2, 512)], lhsT=bsb,
                                     rhs=ones, start=True, stop=False)
                    for k in range(KC):
                        nc.tensor.matmul(
                            out=ps[:, ts(i2, 512)], lhsT=wsb[:, k, :],
                            rhs=xnT[:, k, ds(ih * 1024 + i2 * 512, 512)],
                            start=False, stop=(k == KC - 1))
                nc.vector.tensor_copy(out=dstT[:, cc, ds(ih * 1024, 1024)],
                                      in_=ps)

    def phase_C(xnT, br, vA, seg):
        """v in natural layout [j-part, j-chunk, head, 65] (col 64 = ones)."""
        wvsb = p_w.tile([128, KC, QKCOLS], BF16, tag="wv", name=f"wv_{seg}")
        nc.sync.dma_start(out=wvsb,
                          in_=wv[br].rearrange("(kc p) c -> p kc c", p=128))
        bvsb = p_w.tile([1, QKCOLS], BF16, tag="bv", name=f"bv_{seg}")
        nc.sync.dma_start(out=bvsb, in_=bv[br])
        for j in range(TC):
            ps = ps_mm.tile([128, 1024], F32, tag="mm", name=f"psC_{seg}_{j}")
            nc.tensor.matmul(out=ps[:, 0:512], lhsT=ones[:, 0:128], rhs=bvsb,
                             start=True, stop=False)
            for k in range(KC):
                nc.tensor.matmul(out=ps[:, 0:512], lhsT=xnT[:, k, ts(j, 128)],
                                 rhs=wvsb[:, k, :], start=False,
                                 stop=(k == KC - 1))
            nc.vector.tensor_copy(
                out=vA[:, j, :, 0:DH],
                in_=ps[:, 0:512].rearrange("p (h d) -> p h d", d=DH))

    for ob in range(2):
        sb = 1 - ob
        # prep: q side from branch ob, k/v side from branch sb
        xnT = phase_A(ob, seg=f"{ob}q")
        qT = p_big.tile([128, 4, N], BF16, tag="qT", name=f"qT_{ob}")
        phase_B(xnT, wq[ob], bq[ob], qT, f"q{ob}")
        xnT = phase_A(sb, seg=f"{ob}kv")
        kT = p_big.tile([128, 4, N], BF16, tag="kT", name=f"kT_{ob}")
        phase_B(xnT, wk[sb], bk[sb], kT, f"k{sb}")
        vA = p_big.tile([128, TC, HPC, DH + 1], BF16, tag="vA", name=f"vA_{ob}")
        nc.vector.memset(vA[:, :, :, DH:DH + 1], 1.0)
        phase_C(xnT, sb, vA, seg=f"{ob}")
        aT = p_big.tile([128, 4, N], BF16, tag="aT", name=f"aT_{ob}")

        # ---- attention, one head at a time ----
        # aT receives the UNNORMALIZED output; the per-head Z rows batch
        # into one Ln + one Exp per segment (ACT table sets reload on every
        # Ln<->Exp alternation), then scaling happens in-place on aT.
        zall = p_rz.tile([HPC, N], F32, tag="zall", name=f"zall_{ob}")
        for h in range(HPC):
            pt, po = h // 2, (h % 2) * 64
            avp = ps_av.tile([DH + 1, N], F32, tag="av", name=f"av_{ob}_{h}")
            for j in range(TC):
                es = p_es.tile([128, N], BF16, tag="es", name=f"es_{ob}_{h}_{j}")
                for ih in range(2):
                    ps = ps_mm.tile([128, 1024], F32, tag="mm",
                                    name=f"psS_{ob}_{h}_{j}_{ih}")
                    for i2 in range(2):
                        nc.tensor.matmul(
                            out=ps[:, ts(i2, 512)],
                            lhsT=kT[po:po + 64, pt, ts(j, 128)],
                            rhs=qT[po:po + 64, pt,
                                   ds(ih * 1024 + i2 * 512, 512)],
                            start=True, stop=True)
                    nc.scalar.activation(out=es[:, ts(ih, 1024)], in_=ps,
                                         func=AF.Exp)
                for ib in range(4):
                    nc.tensor.matmul(out=avp[:, ts(ib, 512)],
                                     lhsT=vA[:, j, h, :],
                                     rhs=es[:, ts(ib, 512)],
                                     start=(j == 0), stop=(j == TC - 1))
            # stage unnormalized head output into aT; Z row -> zall[h]
            if po == 0:
                nc.vector.tensor_copy(out=aT[0:64, pt, :], in_=avp[0:64, :])
            else:
                stg = p_rz.tile([64, N], BF16, tag="stg", bufs=2,
                                name=f"stg_{ob}_{h}")
                nc.vector.tensor_copy(out=stg, in_=avp[0:64, :])
                nc.sync.dma_start(out=aT[64:128, pt, :], in_=stg)
            zsb = p_rz.tile([DH + 1, N], F32, tag="zsb", bufs=2,
                            name=f"zsb_{ob}_{h}")
            nc.vector.tensor_copy(out=zsb[64:65, :], in_=avp[64:65, :])
            nc.sync.dma_start(out=zall[h:h + 1, :], in_=zsb[64:65, :])
        # batched recipZ = exp(-ln(Z)) for all 8 heads
        nc.scalar.activation(out=zall, in_=zall, func=AF.Ln)
        nc.scalar.activation(out=zall, in_=zall, func=AF.Exp, scale=-1.0)
        zblk = zst[ob * HPC:(ob + 1) * HPC, :]
        nc.sync.dma_start(out=zblk, in_=zall)
        for h in range(HPC):
            pt, po = h // 2, (h % 2) * 64
            rzb = p_rz.tile([128, N], F32, tag="rzb", bufs=2,
                            name=f"rzb_{ob}_{h}")
            nc.sync.dma_start(
                out=rzb[po:po + 64, :],
                in_=zblk[h:h + 1, :].partition_broadcast(64))
            nc.vector.tensor_mul(out=aT[po:po + 64, pt, :],
                                 in0=aT[po:po + 64, pt, :],
                                 in1=rzb[po:po + 64, :])

        # ---- out-projection (partial over this core's heads) ----
        wosb = p_w.tile([128, 4, DIM], BF16, tag="wo", name=f"wo_{ob}")
        nc.sync.dma_start(out=wosb,
                          in_=wo[ob].rearrange("(hd p) c -> p hd c", p=128))
        for t in range(TC):
            ps = ps_mm.tile([128, 1024], F32, tag="mm", name=f"psE_{ob}_{t}")
            for hd in range(4):
                for cb in range(2):
                    nc.tensor.matmul(out=ps[:, ts(cb, 512)],
                                     lhsT=aT[:, hd, ts(t, 128)],
                                     rhs=wosb[:, hd, ts(cb, 512)],
                                     start=(hd == 0), stop=(hd == 3))
            ot = p_outst.tile([128, DIM], F32, tag="ot", name=f"ot_{ob}_{t}")
            nc.vector.tensor_copy(out=ot, in_=ps)
            nc.sync.dma_start(out=outs[ob][ts(t, 128), :], in_=ot)


_NC = None


def _get_nc():
    global _NC
    if _NC is None:
        _NC = build_program()
    return _NC


def _make_in_maps(x1, x2, ln1_g, ln1_b, ln2_g, ln2_b,
                  w_qkv1, w_qkv2, w_out1, w_out2):
    bf16 = ml_dtypes.bfloat16
    f32 = np.float32
    branches = ((w_qkv1, ln1_g, ln1_b, w_out1), (w_qkv2, ln2_g, ln2_b, w_out2))
    # per head-group g: fold LN affine + softmax scale into weights
    per_g = []
    for g in range(2):
        cols = slice(g * QKCOLS, (g + 1) * QKCOLS)
        m = {}
        for br, (w_qkv, g_ln, b_ln, w_out) in enumerate(branches):
            wq_s = w_qkv[:, 0:DIM][:, cols]
            wk_s = w_qkv[:, DIM:2 * DIM][:, cols]
            wv_s = w_qkv[:, 2 * DIM:3 * DIM][:, cols]
            m[f"wq{br}"] = np.ascontiguousarray(
                (wq_s * g_ln[:, None] * SCALE)).astype(bf16)
            m[f"wk{br}"] = np.ascontiguousarray(wk_s * g_ln[:, None]).astype(bf16)
            m[f"wv{br}"] = np.ascontiguousarray(wv_s * g_ln[:, None]).astype(bf16)
            m[f"bq{br}"] = ((b_ln @ wq_s) * SCALE)[None, :].astype(bf16)
            m[f"bk{br}"] = (b_ln @ wk_s)[None, :].astype(bf16)
            m[f"bv{br}"] = (b_ln @ wv_s)[None, :].astype(bf16)
            m[f"wo{br}"] = np.ascontiguousarray(w_out[cols, :]).astype(bf16)
        per_g.append(m)
    in_maps = []
    for b in range(B):
        for g in range(2):
            m = dict(per_g[g])
            m["x0"] = np.ascontiguousarray(x1[b], dtype=f32)
            m["x1"] = np.ascontiguousarray(x2[b], dtype=f32)
            in_maps.append(m)
    return in_maps


def run(inputs, trace=False):
    """inputs: dict as from setup_inputs(). Returns ((out1, out2), exec_time_ns)."""
    from concourse.bass_utils import run_bass_kernel_spmd

    f32 = np.float32
    ins = {k: np.asarray(v) for k, v in inputs.items()}
    nc = _get_nc()
    in_maps = _make_in_maps(
        ins["x1"].astype(f32), ins["x2"].astype(f32),
        ins["ln1_g"].astype(f32), ins["ln1_b"].astype(f32),
        ins["ln2_g"].astype(f32), ins["ln2_b"].astype(f32),
        ins["w_qkv1"].astype(f32), ins["w_qkv2"].astype(f32),
        ins["w_out1"].astype(f32), ins["w_out2"].astype(f32))
    res = run_bass_kernel_spmd(nc, in_maps, core_ids=list(range(8)), trace=trace)
    r = res.results
    out1 = np.zeros((B, N, DIM), f32)
    out2 = np.zeros((B, N, DIM), f32)
    for b in range(B):
        out1[b] = r[2 * b]["o0"] + r[2 * b + 1]["o0"] + ins["b_out1"].astype(f32)
        out2[b] = r[2 * b]["o1"] + r[2 * b + 1]["o1"] + ins["b_out2"].astype(f32)
    return (out1, out2), res.exec_time_ns


def kernel(**inputs):
    (out1, out2), _ = run(inputs, trace=False)
    return out1, out2



# revision 9
# speedup vs baseline: 1.0583x; 1.0583x over previous
"""Trainium2 Bass kernel for a two-branch cross-attention block.

Math (per branch pair):
    x1n = LN(x1); x2n = LN(x2)
    q1,k1,v1 = split(x1n @ w_qkv1); q2,k2,v2 = split(x2n @ w_qkv2)
    out1 = softmax(q1 k2^T * s) v2 @ w_out1 + b_out1
    out2 = softmax(q2 k1^T * s) v1 @ w_out2 + b_out2

Sharding: 8 cores = 4 batches x 2 head-groups (8 heads each). Each core
handles both branches for its (batch, head-group); the out-projection
contracts over heads, so each core produces a partial [2048, 1024] per
branch and the host sums the two head-group partials + bias.

LN affine (g, b) is folded into the QKV weights on the host; the softmax
scale is folded into the q-side weights. q/k biases are added on the
PSUM->SBUF copy (per-partition scalar), the v bias via a broadcast
tensor-tensor add, so no K=1 bias matmuls reach the PE.

v2 layout (vs the 1.10 ms baseline):
  - AV computed operand-swapped: out[i, d] = es_chunk^T(stationary)
    @ v_aug(moving, 65 cols incl. a ones column for Z). Streams 65
    columns instead of 512 -> AV PE time halved.
  - Attention normalization: recipZ via DVE reciprocal + per-i-chunk
    fused scale on the PSUM->SBUF copy (no DRAM round trip).
  - Single LN per branch feeding all three projections; the branch-0
    LN+transpose is recomputed once as PE filler during attention.
  - Projections / out-projection interleaved as fillers between
    attention heads so the PE keeps running during the ACT-bound
    exp stretches.
"""

import sys
from contextlib import ExitStack

import numpy as np
import ml_dtypes

sys.path.insert(0, "/opt/trn_rl_repo")
sys.path.insert(0, "/opt/trn_rl_repo/concourse")

import concourse.bass as bass
import concourse.tile as tile
from concourse import bacc, mybir
from concourse.bass import ds, ts
from concourse.masks import make_identity

F32 = mybir.dt.float32
BF16 = mybir.dt.bfloat16
AF = mybir.ActivationFunctionType
ALU = mybir.AluOpType

B, N, DIM = 4, 2048, 1024
HEADS, DH = 16, 64
SCALE = DH ** -0.5
HPC = 8          # heads per core
QKCOLS = HPC * DH  # 512 qkv columns per core per tensor
TC = N // 128    # 16 token chunks
KC = DIM // 128  # 8 feature chunks
EPS = 1e-5


def build_program():
    nc = bacc.Bacc(
        "TRN2",
        target_bir_lowering=False,
        debug=False,
        enable_asserts=True,
        num_devices=8,
    )
    xs, wq, wk, wv, bq, bk, bv, wo, outs = [], [], [], [], [], [], [], [], []
    for br in range(2):
        xs.append(nc.dram_tensor(f"x{br}", [N, DIM], F32, kind="ExternalInput").ap())
        wq.append(nc.dram_tensor(f"wq{br}", [DIM, QKCOLS], BF16, kind="ExternalInput").ap())
        wk.append(nc.dram_tensor(f"wk{br}", [DIM, QKCOLS], BF16, kind="ExternalInput").ap())
        wv.append(nc.dram_tensor(f"wv{br}", [DIM, QKCOLS], BF16, kind="ExternalInput").ap())
        bq.append(nc.dram_tensor(f"bq{br}", [128, 4], F32, kind="ExternalInput").ap())
        bk.append(nc.dram_tensor(f"bk{br}", [128, 4], F32, kind="ExternalInput").ap())
        bv.append(nc.dram_tensor(f"bv{br}", [1, QKCOLS], BF16, kind="ExternalInput").ap())
        wo.append(nc.dram_tensor(f"wo{br}", [QKCOLS, DIM], BF16, kind="ExternalInput").ap())
        outs.append(nc.dram_tensor(f"o{br}", [N, DIM], F32, kind="ExternalOutput").ap())

    with tile.TileContext(nc) as tc:
        with ExitStack() as ctx:
            _body(ctx, tc, xs, wq, wk, wv, bq, bk, bv, wo, outs)
    nc.finalize()
    return nc


def _body(ctx, tc, xs, wq, wk, wv, bq, bk, bv, wo, outs):
    nc = tc.nc
    p_const = ctx.enter_context(tc.tile_pool(name="const", bufs=1))
    p_x = ctx.enter_context(tc.tile_pool(name="x", bufs=2))
    p_stat = ctx.enter_context(tc.tile_pool(name="stat", bufs=1))
    p_z = ctx.enter_context(tc.tile_pool(name="z", bufs=2))
    p_w = ctx.enter_context(tc.tile_pool(name="w", bufs=1))
    p_big = ctx.enter_context(tc.tile_pool(name="big", bufs=1))
    p_es = ctx.enter_context(tc.tile_pool(name="es", bufs=2))
    p_outst = ctx.enter_context(tc.tile_pool(name="outst", bufs=1))
    ps_mm = ctx.enter_context(tc.tile_pool(name="ps_mm", bufs=2, space="PSUM"))
    ps_av = ctx.enter_context(tc.tile_pool(name="ps_av", bufs=1, space="PSUM"))

    ident = p_const.tile([128, 128], BF16, tag="ident", name="ident")
    make_identity(nc, ident)

    # per-branch LN stats kept alive for the branch-0 recompute
    stats = [None, None]
    rstd = [None, None]
    nmr = [None, None]

    def ln_stats(br):
        """Pass 1 over x[br]: bn stats, rstd, -mu*rstd (all [128, TC])."""
        st_all = p_stat.tile([128, TC, 2], F32, tag=f"stats{br}",
                             name=f"stats_{br}")
        rs = p_stat.tile([128, TC], F32, tag=f"rstd{br}", name=f"rstd_{br}")
        nm = p_stat.tile([128, TC], F32, tag=f"nmr{br}", name=f"nmr_{br}")
        epst = p_stat.tile([128, 1], F32, tag=f"eps{br}", name=f"epst_{br}")
        nc.vector.memset(epst, EPS)
        for t in range(TC):
            xt = p_x.tile([128, DIM], F32, tag="xt", name=f"xs{br}_{t}")
            nc.sync.dma_start(out=xt, in_=xs[br][ts(t, 128), :])
            st = p_stat.tile([128, 2, 6], F32, tag="st", name=f"st{br}_{t}")
            for sg in range(2):
                nc.vector.bn_stats(out=st[:, sg, :], in_=xt[:, ts(sg, 512)])
            nc.vector.bn_aggr(out=st_all[:, t, :], in_=st)
        # rstd = exp(-0.5 * ln(var + eps)), batched over all 16 chunks
        nc.scalar.activation(out=rs, in_=st_all[:, :, 1], func=AF.Ln,
                             bias=epst, scale=1.0)
        nc.scalar.activation(out=rs, in_=rs, func=AF.Exp, scale=-0.5)
        # nmr = -mu * rstd (bias for the ACT/DVE normalize apply)
        nc.vector.tensor_tensor(out=nm, in0=st_all[:, :, 0], in1=rs,
                                op=ALU.mult)
        nc.vector.tensor_scalar(out=nm, in0=nm, scalar1=-1.0, scalar2=None,
                                op0=ALU.mult)
        stats[br], rstd[br], nmr[br] = st_all, rs, nm

    def ln_apply_unit(br, t, xnT, seg, on_act):
        """Pass 2, one token chunk: z = x*rstd - mu*rstd (bf16), transpose
        into xnT[:, :, t*128:(t+1)*128]."""
        xt = p_x.tile([128, DIM], F32, tag="xt", name=f"xa{seg}_{t}")
        nc.sync.dma_start(out=xt, in_=xs[br][ts(t, 128), :])
        zt = p_z.tile([128, DIM], BF16, tag="zt", name=f"zt{seg}_{t}")
        if on_act:
            nc.scalar.activation(out=zt, in_=xt, func=AF.Identity,
                                 bias=nmr[br][:, t:t + 1],
                                 scale=rstd[br][:, t:t + 1])
        else:
            nc.vector.tensor_scalar(out=zt, in0=xt,
                                    scalar1=stats[br][:, t, 0:1],
                                    scalar2=rstd[br][:, t:t + 1],
                                    op0=ALU.subtract, op1=ALU.mult)
        ptr = ps_mm.tile([128, KC, 128], BF16, tag="mm", name=f"ptr{seg}_{t}")
        for fc in range(KC):
            nc.tensor.transpose(out=ptr[:, fc, :], in_=zt[:, ts(fc, 128)],
                                identity=ident)
        nc.vector.tensor_copy(out=xnT[:, :, ts(t, 128)], in_=ptr)

    def load_w_re(wt_d, lbl):
        wsb = p_w.tile([128, KC, QKCOLS], BF16, tag="w", bufs=1,
                       name=f"w_{lbl}")
        nc.sync.dma_start(out=wsb,
                          in_=wt_d.rearrange("(kc p) c -> p kc c", p=128))
        return wsb

    def load_bias(bias_d, lbl):
        bsb = p_stat.tile([128, 4], F32, tag=f"b_{lbl}", name=f"b_{lbl}")
        nc.sync.dma_start(out=bsb, in_=bias_d)
        return bsb

    def proj_qk_unit(xnT, wsb, bsb, dstT, cc, ih, i2, lbl):
        """One [128 cols x 512 tokens] chunk of a q/k projection,
        transposed output layout, bias fused into the copy."""
        ps = ps_mm.tile([128, 512], F32, tag="mm",
                        name=f"psB_{lbl}_{cc}_{ih}_{i2}")
        for k in range(KC):
            nc.tensor.matmul(
                out=ps, lhsT=wsb[:, k, ts(cc, 128)],
                rhs=xnT[:, k, ds(ih * 1024 + i2 * 512, 512)],
                start=(k == 0), stop=(k == KC - 1))
        nc.vector.tensor_scalar(out=dstT[:, cc, ds(ih * 1024 + i2 * 512, 512)],
                                in0=ps, scalar1=bsb[:, cc:cc + 1],
                                scalar2=None, op0=ALU.add)

    def proj_v_unit(xnT, wsb, bvb, vA, jc, lbl):
        """v for one j-chunk, natural layout [j, h, d], bias row added."""
        ps = ps_mm.tile([128, 1024], F32, tag="mm", name=f"psC_{lbl}_{jc}")
        for k in range(KC):
            nc.tensor.matmul(out=ps[:, 0:512], lhsT=xnT[:, k, ts(jc, 128)],
                             rhs=wsb[:, k, :], start=(k == 0),
                             stop=(k == KC - 1))
        nc.vector.tensor_tensor(
            out=vA[:, jc, :, 0:DH],
            in0=ps[:, 0:512].rearrange("p (h d) -> p h d", d=DH),
            in1=bvb.rearrange("p (h d) -> p h d", d=DH), op=ALU.add)

    def outproj_unit(aT, wosb, ob, t):
        ps = ps_mm.tile([128, 1024], F32, tag="mm", name=f"psE_{ob}_{t}")
        for hd in range(4):
            for cb in range(2):
                nc.tensor.matmul(out=ps[:, ts(cb, 512)],
                                 lhsT=aT[:, hd, ts(t, 128)],
                                 rhs=wosb[:, hd, ts(cb, 512)],
                                 start=(hd == 0), stop=(hd == 3))
        ot = p_outst.tile([128, DIM], F32, tag="ot", name=f"ot_{ob}_{t}")
        nc.vector.tensor_copy(out=ot, in_=ps)
        nc.sync.dma_start(out=outs[ob][ts(t, 128), :], in_=ot)

    # ---- persistent big tiles ----
    xnT = p_big.tile([128, KC, N], BF16, tag="xnT", name="xnT")
    qT = [p_big.tile([128, 4, N], BF16, tag=f"qT{br}", name=f"qT_{br}")
          for br in range(2)]
    kT = [p_big.tile([128, 4, N], BF16, tag=f"kT{br}", name=f"kT_{br}")
          for br in range(2)]
    vA = [p_big.tile([128, TC, HPC, DH + 1], BF16, tag=f"vA{br}",
                     name=f"vA_{br}") for br in range(2)]
    aT = p_big.tile([128, 4, N], BF16, tag="aT", name="aT")
    attn = p_big.tile([128, TC, HPC, DH], BF16, tag="attn", name="attn")
    rz = p_big.tile([128, TC, HPC], F32, tag="rz", name="rz")
    for br in range(2):
        nc.vector.memset(vA[br][:, :, :, DH:DH + 1], 1.0)

    bvb = [None, None]
    for br in range(2):
        bvb[br] = p_const.tile([128, QKCOLS], BF16, tag=f"bvb{br}",
                               name=f"bvb_{br}")
        nc.sync.dma_start(out=bvb[br], in_=bv[br].partition_broadcast(128))

    def attention(ob, fillers, pump):
        """ob: 0 -> q from branch 0, k/v from branch 1; 1 -> reverse."""
        sb = 1 - ob
        my_q, my_k, my_v = qT[ob], kT[sb], vA[sb]
        fi = [0]  # filler cursor

        def do_fill(n):
            for _ in range(n):
                if fi[0] < len(fillers):
                    fillers[fi[0]]()
                    fi[0] += 1

        def av_round(av, es, jc, h):
            # PSUM accumulation groups are per 2KB bank: av bank b holds
            # i-chunks 4b..4b+3, so start/stop only on the bank's first/last
            # write of the whole jc loop.
            for ic in range(TC):
                nc.tensor.matmul(out=av[:, ic, 0:DH + 1],
                                 lhsT=es[:, ts(ic, 128)],
                                 rhs=my_v[:, jc, h, :],
                                 start=(jc == 0 and ic % 4 == 0),
                                 stop=(jc == TC - 1 and ic % 4 == 3))

        for h in range(HPC):
            pt, po = h // 2, (h % 2) * 64
            av = ps_av.tile([128, TC, 128], F32, tag="av",
                            name=f"av_{ob}_{h}")
            prev = None
            for jc in range(TC):
                es = p_es.tile([128, N], BF16, tag="es",
                               name=f"es_{ob}_{h}_{jc}")
                for ih in range(2):
                    ps = ps_mm.tile([128, 1024], F32, tag="mm",
                                    name=f"psS_{ob}_{h}_{jc}_{ih}")
                    for i2 in range(2):
                        nc.tensor.matmul(
                            out=ps[:, ts(i2, 512)],
                            lhsT=my_k[po:po + 64, pt, ts(jc, 128)],
                            rhs=my_q[po:po + 64, pt,
                                     ds(ih * 1024 + i2 * 512, 512)],
                            start=True, stop=True)
                    nc.scalar.activation(out=es[:, ts(ih, 1024)], in_=ps,
                                         func=AF.Exp)
                # software pipeline: AV for jc-1 lands after S(jc) so the
                # PE isn't parked behind the exp it needs
                if prev is not None:
                    av_round(av, prev, jc - 1, h)
                    if jc % 2 == 0 and pump:
                        do_fill(1)
                prev = es
            av_round(av, prev, TC - 1, h)
            # epilogue: recipZ, fused normalize into attn staging
            nc.vector.reciprocal(out=rz[:, :, h:h + 1], in_=av[:, :, DH:DH + 1])
            for ic in range(TC):
                nc.vector.tensor_scalar(
                    out=attn[:, ic, h, :], in0=av[:, ic, 0:DH],
                    scalar1=rz[:, ic, h:h + 1], scalar2=None, op0=ALU.mult)
            do_fill(1)
        # drain remaining fillers
        do_fill(len(fillers))
        # transpose attn [i, (h d)] -> aT [(h d), i]
        for ic in range(TC):
            tp = ps_mm.tile([128, 4, 128], BF16, tag="mm", name=f"tp{ob}_{ic}")
            for b4 in range(4):
                nc.tensor.transpose(out=tp[:, b4, :],
                                    in_=attn[:, ic, ds(2 * b4, 2), :],
                                    identity=ident)
            nc.vector.tensor_copy(out=aT[:, :, ts(ic, 128)], in_=tp)

    # ================= prologue =================
    ln_stats(0)
    for t in range(TC):
        ln_apply_unit(0, t, xnT, "p0", on_act=True)
    wq0 = load_w_re(wq[0], "q0")
    bq0 = load_bias(bq[0], "q0")
    ln_stats(1)  # DVE stats overlap the q1 projection below
    for cc in range(4):
        for ih in range(2):
            for i2 in range(2):
                proj_qk_unit(xnT, wq0, bq0, qT[0], cc, ih, i2, "q0")
    for t in range(TC):
        ln_apply_unit(1, t, xnT, "p1", on_act=True)
    wk1 = load_w_re(wk[1], "k1")
    bk1 = load_bias(bk[1], "k1")
    for cc in range(4):
        for ih in range(2):
            for i2 in range(2):
                proj_qk_unit(xnT, wk1, bk1, kT[1], cc, ih, i2, "k1")
    wv1 = load_w_re(wv[1], "v1")
    for jc in range(TC):
        proj_v_unit(xnT, wv1, bvb[1], vA[1], jc, "v1")
    wq1 = load_w_re(wq[1], "q1")
    bq1 = load_bias(bq[1], "q1")
    for cc in range(4):
        for ih in range(2):
            for i2 in range(2):
                proj_qk_unit(xnT, wq1, bq1, qT[1], cc, ih, i2, "q1")

    # ============ attention ob=0 with fillers ============
    # fillers: recompute branch-0 LN/transpose (DVE z-apply), then k1/v1
    fillers0 = []
    for t in range(TC):
        fillers0.append(
            lambda t=t: ln_apply_unit(0, t, xnT, "r0", on_act=False))
    wk0 = load_w_re(wk[0], "k0")
    bk0 = load_bias(bk[0], "k0")
    for cc in range(4):
        for ih in range(2):
            for i2 in range(2):
                fillers0.append(
                    lambda cc=cc, ih=ih, i2=i2: proj_qk_unit(
                        xnT, wk0, bk0, kT[0], cc, ih, i2, "k0"))
    wv0 = load_w_re(wv[0], "v0")
    for jc in range(TC):
        fillers0.append(
            lambda jc=jc: proj_v_unit(xnT, wv0, bvb[0], vA[0], jc, "v0"))
    attention(0, fillers0, pump=1)

    # ============ attention ob=1 with fillers ============
    wo0 = p_w.tile([128, 4, DIM], BF16, tag="wo", name="wo_0")
    nc.sync.dma_start(out=wo0, in_=wo[0].rearrange("(hd p) c -> p hd c", p=128))
    fillers1 = [lambda t=t: outproj_unit(aT, wo0, 0, t) for t in range(TC)]
    attention(1, fillers1, pump=1)

    wo1 = p_w.tile([128, 4, DIM], BF16, tag="wo", name="wo_1")
    nc.sync.dma_start(out=wo1, in_=wo[1].rearrange("(hd p) c -> p hd c", p=128))
    for t in range(TC):
        outproj_unit(aT, wo1, 1, t)


_NC = None


def _get_nc():
    global _NC
    if _NC is None:
        _NC = build_program()
    return _NC


def _make_in_maps(x1, x2, ln1_g, ln1_b, ln2_g, ln2_b,
                  w_qkv1, w_qkv2, w_out1, w_out2):
    bf16 = ml_dtypes.bfloat16
    f32 = np.float32
    branches = ((w_qkv1, ln1_g, ln1_b, w_out1), (w_qkv2, ln2_g, ln2_b, w_out2))
    # per head-group g: fold LN affine + softmax scale into weights
    per_g = []
    for g in range(2):
        cols = slice(g * QKCOLS, (g + 1) * QKCOLS)
        m = {}
        for br, (w_qkv, g_ln, b_ln, w_out) in enumerate(branches):
            wq_s = w_qkv[:, 0:DIM][:, cols]
            wk_s = w_qkv[:, DIM:2 * DIM][:, cols]
            wv_s = w_qkv[:, 2 * DIM:3 * DIM][:, cols]
            m[f"wq{br}"] = np.ascontiguousarray(
                (wq_s * g_ln[:, None] * SCALE)).astype(bf16)
            m[f"wk{br}"] = np.ascontiguousarray(wk_s * g_ln[:, None]).astype(bf16)
            m[f"wv{br}"] = np.ascontiguousarray(wv_s * g_ln[:, None]).astype(bf16)
            # q/k biases as [128, 4] per-partition columns (col cc, part p
            # holds bias for q-col cc*128+p)
            m[f"bq{br}"] = np.ascontiguousarray(
                ((b_ln @ wq_s) * SCALE).reshape(4, 128).T).astype(f32)
            m[f"bk{br}"] = np.ascontiguousarray(
                (b_ln @ wk_s).reshape(4, 128).T).astype(f32)
            m[f"bv{br}"] = (b_ln @ wv_s)[None, :].astype(bf16)
            m[f"wo{br}"] = np.ascontiguousarray(w_out[cols, :]).astype(bf16)
        per_g.append(m)
    in_maps = []
    for b in range(B):
        for g in range(2):
            m = dict(per_g[g])
            m["x0"] = np.ascontiguousarray(x1[b], dtype=f32)
            m["x1"] = np.ascontiguousarray(x2[b], dtype=f32)
            in_maps.append(m)
    return in_maps


def run(inputs, trace=False):
    """inputs: dict as from setup_inputs(). Returns ((out1, out2), exec_time_ns)."""
    from concourse.bass_utils import run_bass_kernel_spmd

    f32 = np.float32
    ins = {k: np.asarray(v) for k, v in inputs.items()}
    nc = _get_nc()
    in_maps = _make_in_maps(
        ins["x1"].astype(f32), ins["x2"].astype(f32),
        ins["ln1_g"].astype(f32), ins["ln1_b"].astype(f32),
        ins["ln2_g"].astype(f32), ins["ln2_b"].astype(f32),
        ins["w_qkv1"].astype(f32), ins["w_qkv2"].astype(f32),
        ins["w_out1"].astype(f32), ins["w_out2"].astype(f32))
    res = run_bass_kernel_spmd(nc, in_maps, core_ids=list(range(8)), trace=trace)
    r = res.results
    out1 = np.zeros((B, N, DIM), f32)
    out2 = np.zeros((B, N, DIM), f32)
    for b in range(B):
        out1[b] = r[2 * b]["o0"] + r[2 * b + 1]["o0"] + ins["b_out1"].astype(f32)
        out2[b] = r[2 * b]["o1"] + r[2 * b + 1]["o1"] + ins["b_out2"].astype(f32)
    return (out1, out2), res.exec_time_ns


def kernel(**inputs):
    (out1, out2), _ = run(inputs, trace=False)
    return out1, out2


# revision 10
# speedup vs baseline: 1.0920x; 1.0318x over previous
"""Trainium2 Bass kernel for a two-branch cross-attention block.

Math (per branch pair):
    x1n = LN(x1); x2n = LN(x2)
    q1,k1,v1 = split(x1n @ w_qkv1); q2,k2,v2 = split(x2n @ w_qkv2)
    out1 = softmax(q1 k2^T * s) v2 @ w_out1 + b_out1
    out2 = softmax(q2 k1^T * s) v1 @ w_out2 + b_out2

Sharding: 8 cores = 4 batches x 2 head-groups (8 heads each). Each core
handles both branches for its (batch, head-group); the out-projection
contracts over heads, so each core produces a partial [2048, 1024] per
branch and the host sums the two head-group partials + bias.

LN affine (g, b) is folded into the QKV weights on the host; the softmax
scale is folded into the q-side weights. q/k biases are added on the
PSUM->SBUF copy (per-partition scalar), the v bias via a broadcast
tensor-tensor add, so no K=1 bias matmuls reach the PE.

v2 layout (vs the 1.10 ms baseline):
  - AV computed operand-swapped: out[i, d] = es_chunk^T(stationary)
    @ v_aug(moving, 65 cols incl. a ones column for Z). Streams 65
    columns instead of 512 -> AV PE time halved.
  - Attention normalization: recipZ via DVE reciprocal + per-i-chunk
    fused scale on the PSUM->SBUF copy (no DRAM round trip).
  - Single LN per branch feeding all three projections; the branch-0
    LN+transpose is recomputed once as PE filler during attention.
  - Projections / out-projection interleaved as fillers between
    attention heads so the PE keeps running during the ACT-bound
    exp stretches.
"""

import sys
from contextlib import ExitStack

import numpy as np
import ml_dtypes

sys.path.insert(0, "/opt/trn_rl_repo")
sys.path.insert(0, "/opt/trn_rl_repo/concourse")

import concourse.bass as bass
import concourse.tile as tile
from concourse import bacc, mybir
from concourse.bass import ds, ts
from concourse.masks import make_identity

F32 = mybir.dt.float32
BF16 = mybir.dt.bfloat16
AF = mybir.ActivationFunctionType
ALU = mybir.AluOpType

B, N, DIM = 4, 2048, 1024
HEADS, DH = 16, 64
SCALE = DH ** -0.5
HPC = 8          # heads per core
QKCOLS = HPC * DH  # 512 qkv columns per core per tensor
TC = N // 128    # 16 token chunks
KC = DIM // 128  # 8 feature chunks
EPS = 1e-5


def build_program():
    nc = bacc.Bacc(
        "TRN2",
        target_bir_lowering=False,
        debug=False,
        enable_asserts=True,
        num_devices=8,
    )
    xs, wq, wk, wv, bq, bk, bv, wo, outs = [], [], [], [], [], [], [], [], []
    for br in range(2):
        xs.append(nc.dram_tensor(f"x{br}", [N, DIM], F32, kind="ExternalInput").ap())
        wq.append(nc.dram_tensor(f"wq{br}", [DIM, QKCOLS], BF16, kind="ExternalInput").ap())
        wk.append(nc.dram_tensor(f"wk{br}", [DIM, QKCOLS], BF16, kind="ExternalInput").ap())
        wv.append(nc.dram_tensor(f"wv{br}", [DIM, QKCOLS], BF16, kind="ExternalInput").ap())
        bq.append(nc.dram_tensor(f"bq{br}", [128, 4], F32, kind="ExternalInput").ap())
        bk.append(nc.dram_tensor(f"bk{br}", [128, 4], F32, kind="ExternalInput").ap())
        bv.append(nc.dram_tensor(f"bv{br}", [1, QKCOLS], BF16, kind="ExternalInput").ap())
        wo.append(nc.dram_tensor(f"wo{br}", [QKCOLS, DIM], BF16, kind="ExternalInput").ap())
        outs.append(nc.dram_tensor(f"o{br}", [N, DIM], F32, kind="ExternalOutput").ap())

    with tile.TileContext(nc) as tc:
        with ExitStack() as ctx:
            _body(ctx, tc, xs, wq, wk, wv, bq, bk, bv, wo, outs)
    nc.finalize()
    return nc


def _body(ctx, tc, xs, wq, wk, wv, bq, bk, bv, wo, outs):
    nc = tc.nc
    p_const = ctx.enter_context(tc.tile_pool(name="const", bufs=1))
    p_x = ctx.enter_context(tc.tile_pool(name="x", bufs=2))
    p_stat = ctx.enter_context(tc.tile_pool(name="stat", bufs=1))
    p_z = ctx.enter_context(tc.tile_pool(name="z", bufs=2))
    p_w = ctx.enter_context(tc.tile_pool(name="w", bufs=1))
    p_big = ctx.enter_context(tc.tile_pool(name="big", bufs=1))
    p_es = ctx.enter_context(tc.tile_pool(name="es", bufs=2))
    p_outst = ctx.enter_context(tc.tile_pool(name="outst", bufs=1))
    # PSUM budget (8 banks): S double-buffer 2x2, AV accumulator 2 (i in
    # halves per head), fillers/transposes/projections 2.
    ps_s = ctx.enter_context(tc.tile_pool(name="ps_s", bufs=2, space="PSUM"))
    ps_mm = ctx.enter_context(tc.tile_pool(name="ps_mm", bufs=1, space="PSUM"))
    ps_av = ctx.enter_context(tc.tile_pool(name="ps_av", bufs=1, space="PSUM"))

    ident = p_const.tile([128, 128], BF16, tag="ident", name="ident")
    make_identity(nc, ident)

    # per-branch LN stats kept alive for the branch-0 recompute
    stats = [None, None]
    rstd = [None, None]
    nmr = [None, None]

    def ln_stats(br):
        """Pass 1 over x[br]: bn stats, rstd, -mu*rstd (all [128, TC])."""
        st_all = p_stat.tile([128, TC, 2], F32, tag=f"stats{br}",
                             name=f"stats_{br}")
        rs = p_stat.tile([128, TC], F32, tag=f"rstd{br}", name=f"rstd_{br}")
        nm = p_stat.tile([128, TC], F32, tag=f"nmr{br}", name=f"nmr_{br}")
        epst = p_stat.tile([128, 1], F32, tag=f"eps{br}", name=f"epst_{br}")
        nc.vector.memset(epst, EPS)
        for t in range(TC):
            xt = p_x.tile([128, DIM], F32, tag="xt", name=f"xs{br}_{t}")
            nc.sync.dma_start(out=xt, in_=xs[br][ts(t, 128), :])
            st = p_stat.tile([128, 2, 6], F32, tag="st", name=f"st{br}_{t}")
            for sg in range(2):
                nc.vector.bn_stats(out=st[:, sg, :], in_=xt[:, ts(sg, 512)])
            nc.vector.bn_aggr(out=st_all[:, t, :], in_=st)
        # rstd = exp(-0.5 * ln(var + eps)), batched over all 16 chunks
        nc.scalar.activation(out=rs, in_=st_all[:, :, 1], func=AF.Ln,
                             bias=epst, scale=1.0)
        nc.scalar.activation(out=rs, in_=rs, func=AF.Exp, scale=-0.5)
        # nmr = -mu * rstd (bias for the ACT/DVE normalize apply)
        nc.vector.tensor_tensor(out=nm, in0=st_all[:, :, 0], in1=rs,
                                op=ALU.mult)
        nc.vector.tensor_scalar(out=nm, in0=nm, scalar1=-1.0, scalar2=None,
                                op0=ALU.mult)
        stats[br], rstd[br], nmr[br] = st_all, rs, nm

    def ln_apply_unit(br, t, xnT, seg, on_act):
        """Pass 2, one token chunk: z = x*rstd - mu*rstd (bf16), transpose
        into xnT[:, :, t*128:(t+1)*128]."""
        xt = p_x.tile([128, DIM], F32, tag="xt", name=f"xa{seg}_{t}")
        nc.sync.dma_start(out=xt, in_=xs[br][ts(t, 128), :])
        zt = p_z.tile([128, DIM], BF16, tag="zt", name=f"zt{seg}_{t}")
        if on_act:
            nc.scalar.activation(out=zt, in_=xt, func=AF.Identity,
                                 bias=nmr[br][:, t:t + 1],
                                 scale=rstd[br][:, t:t + 1])
        else:
            nc.vector.tensor_scalar(out=zt, in0=xt,
                                    scalar1=stats[br][:, t, 0:1],
                                    scalar2=rstd[br][:, t:t + 1],
                                    op0=ALU.subtract, op1=ALU.mult)
        ptr = ps_mm.tile([128, KC, 128], BF16, tag="mm", name=f"ptr{seg}_{t}")
        for fc in range(KC):
            nc.tensor.transpose(out=ptr[:, fc, :], in_=zt[:, ts(fc, 128)],
                                identity=ident)
        nc.vector.tensor_copy(out=xnT[:, :, ts(t, 128)], in_=ptr)

    def load_w_re(wt_d, lbl):
        wsb = p_w.tile([128, KC, QKCOLS], BF16, tag="w", bufs=1,
                       name=f"w_{lbl}")
        nc.sync.dma_start(out=wsb,
                          in_=wt_d.rearrange("(kc p) c -> p kc c", p=128))
        return wsb

    def load_bias(bias_d, lbl):
        bsb = p_stat.tile([128, 4], F32, tag=f"b_{lbl}", name=f"b_{lbl}")
        nc.sync.dma_start(out=bsb, in_=bias_d)
        return bsb

    def proj_qk_unit(xnT, wsb, bsb, dstT, cc, ih, i2, lbl):
        """One [128 cols x 512 tokens] chunk of a q/k projection,
        transposed output layout, bias fused into the copy."""
        ps = ps_mm.tile([128, 512], F32, tag="mm",
                        name=f"psB_{lbl}_{cc}_{ih}_{i2}")
        for k in range(KC):
            nc.tensor.matmul(
                out=ps, lhsT=wsb[:, k, ts(cc, 128)],
                rhs=xnT[:, k, ds(ih * 1024 + i2 * 512, 512)],
                start=(k == 0), stop=(k == KC - 1))
        nc.vector.tensor_scalar(out=dstT[:, cc, ds(ih * 1024 + i2 * 512, 512)],
                                in0=ps, scalar1=bsb[:, cc:cc + 1],
                                scalar2=None, op0=ALU.add)

    def proj_v_unit(xnT, wsb, bvb, vA, jc, lbl):
        """v for one j-chunk, natural layout [j, h, d], bias row added."""
        ps = ps_mm.tile([128, 1024], F32, tag="mm", name=f"psC_{lbl}_{jc}")
        for k in range(KC):
            nc.tensor.matmul(out=ps[:, 0:512], lhsT=xnT[:, k, ts(jc, 128)],
                             rhs=wsb[:, k, :], start=(k == 0),
                             stop=(k == KC - 1))
        nc.vector.tensor_tensor(
            out=vA[:, jc, :, 0:DH],
            in0=ps[:, 0:512].rearrange("p (h d) -> p h d", d=DH),
            in1=bvb.rearrange("p (h d) -> p h d", d=DH), op=ALU.add)

    def outproj_unit(aT, wosb, ob, t):
        ps = ps_mm.tile([128, 1024], F32, tag="mm", name=f"psE_{ob}_{t}")
        for hd in range(4):
            for cb in range(2):
                nc.tensor.matmul(out=ps[:, ts(cb, 512)],
                                 lhsT=aT[:, hd, ts(t, 128)],
                                 rhs=wosb[:, hd, ts(cb, 512)],
                                 start=(hd == 0), stop=(hd == 3))
        ot = p_outst.tile([128, DIM], F32, tag="ot", name=f"ot_{ob}_{t}")
        nc.vector.tensor_copy(out=ot, in_=ps)
        nc.sync.dma_start(out=outs[ob][ts(t, 128), :], in_=ot)

    # ---- persistent big tiles ----
    xnT = p_big.tile([128, KC, N], BF16, tag="xnT", name="xnT")
    qT = [p_big.tile([128, 4, N], BF16, tag=f"qT{br}", name=f"qT_{br}")
          for br in range(2)]
    kT = [p_big.tile([128, 4, N], BF16, tag=f"kT{br}", name=f"kT_{br}")
          for br in range(2)]
    vA = [p_big.tile([128, TC, HPC, DH + 1], BF16, tag=f"vA{br}",
                     name=f"vA_{br}") for br in range(2)]
    aT = p_big.tile([128, 4, N], BF16, tag="aT", name="aT")
    attn = p_big.tile([128, TC, HPC, DH], BF16, tag="attn", name="attn")
    rz = p_big.tile([128, TC, HPC], F32, tag="rz", name="rz")
    for br in range(2):
        nc.vector.memset(vA[br][:, :, :, DH:DH + 1], 1.0)

    bvb = [None, None]
    for br in range(2):
        bvb[br] = p_const.tile([128, QKCOLS], BF16, tag=f"bvb{br}",
                               name=f"bvb_{br}")
        nc.sync.dma_start(out=bvb[br], in_=bv[br].partition_broadcast(128))

    def attention(ob, fillers, pump):
        """ob: 0 -> q from branch 0, k/v from branch 1; 1 -> reverse."""
        sb = 1 - ob
        my_q, my_k, my_v = qT[ob], kT[sb], vA[sb]
        fi = [0]  # filler cursor

        def do_fill(n):
            for _ in range(n):
                if fi[0] < len(fillers):
                    fillers[fi[0]]()
                    fi[0] += 1

        pend = [None]  # deferred AV round: (av, es, jc, h)

        def av_flush():
            if pend[0] is None:
                return
            av, es, jc, h = pend[0]
            pend[0] = None
            # PSUM accumulation groups are per 2KB bank: av bank b holds
            # i-chunks 4b..4b+3, so start/stop only on the bank's first/last
            # write of the whole jc loop.
            for ic in range(8):
                nc.tensor.matmul(out=av[:, ic, 0:DH + 1],
                                 lhsT=es[:, ts(ic, 128)],
                                 rhs=my_v[:, jc, h, :],
                                 start=(jc == 0 and ic % 4 == 0),
                                 stop=(jc == TC - 1 and ic % 4 == 3))

        jcount = [0]
        for h in range(HPC):
            pt, po = h // 2, (h % 2) * 64
            for ihalf in range(2):
                av = ps_av.tile([128, 8, 128], F32, tag="av",
                                name=f"av_{ob}_{h}_{ihalf}")
                for jc in range(TC):
                    es = p_es.tile([128, 1024], BF16, tag="es",
                                   name=f"es_{ob}_{h}_{ihalf}_{jc}")
                    ps = ps_s.tile([128, 1024], F32, tag="s",
                                   name=f"psS_{ob}_{h}_{ihalf}_{jc}")
                    for i2 in range(2):
                        nc.tensor.matmul(
                            out=ps[:, ts(i2, 512)],
                            lhsT=my_k[po:po + 64, pt, ts(jc, 128)],
                            rhs=my_q[po:po + 64, pt,
                                     ds(ihalf * 1024 + i2 * 512, 512)],
                            start=True, stop=True)
                    nc.scalar.activation(out=es, in_=ps, func=AF.Exp)
                    # software pipeline: previous AV lands after this S so
                    # the PE isn't parked behind the exp it needs
                    av_flush()
                    pend[0] = (av, es, jc, h)
                    jcount[0] += 1
                    if jcount[0] % 3 == 0:
                        do_fill(1)
                av_flush()
                # epilogue: recipZ, fused normalize into attn staging
                ib = ihalf * 8
                nc.vector.reciprocal(out=rz[:, ds(ib, 8), h:h + 1],
                                     in_=av[:, :, DH:DH + 1])
                for ic in range(8):
                    nc.vector.tensor_scalar(
                        out=attn[:, ib + ic, h, :], in0=av[:, ic, 0:DH],
                        scalar1=rz[:, ib + ic, h:h + 1], scalar2=None,
                        op0=ALU.mult)
        av_flush()
        # drain remaining fillers
        do_fill(len(fillers))
        # transpose attn [i, (h d)] -> aT [(h d), i]
        for ic in range(TC):
            tp = ps_mm.tile([128, 4, 128], BF16, tag="mm", name=f"tp{ob}_{ic}")
            for b4 in range(4):
                nc.tensor.transpose(out=tp[:, b4, :],
                                    in_=attn[:, ic, ds(2 * b4, 2), :],
                                    identity=ident)
            nc.vector.tensor_copy(out=aT[:, :, ts(ic, 128)], in_=tp)

    # ================= prologue =================
    ln_stats(0)
    for t in range(TC):
        ln_apply_unit(0, t, xnT, "p0", on_act=True)
    wq0 = load_w_re(wq[0], "q0")
    bq0 = load_bias(bq[0], "q0")
    ln_stats(1)  # DVE stats overlap the q1 projection below
    for cc in range(4):
        for ih in range(2):
            for i2 in range(2):
                proj_qk_unit(xnT, wq0, bq0, qT[0], cc, ih, i2, "q0")
    for t in range(TC):
        ln_apply_unit(1, t, xnT, "p1", on_act=True)
    wk1 = load_w_re(wk[1], "k1")
    bk1 = load_bias(bk[1], "k1")
    for cc in range(4):
        for ih in range(2):
            for i2 in range(2):
                proj_qk_unit(xnT, wk1, bk1, kT[1], cc, ih, i2, "k1")
    wv1 = load_w_re(wv[1], "v1")
    for jc in range(TC):
        proj_v_unit(xnT, wv1, bvb[1], vA[1], jc, "v1")
    wq1 = load_w_re(wq[1], "q1")
    bq1 = load_bias(bq[1], "q1")
    for cc in range(4):
        for ih in range(2):
            for i2 in range(2):
                proj_qk_unit(xnT, wq1, bq1, qT[1], cc, ih, i2, "q1")

    # ============ attention ob=0 with fillers ============
    # fillers: recompute branch-0 LN/transpose (DVE z-apply), then k1/v1
    fillers0 = []
    for t in range(TC):
        fillers0.append(
            lambda t=t: ln_apply_unit(0, t, xnT, "r0", on_act=False))
    wk0 = load_w_re(wk[0], "k0")
    bk0 = load_bias(bk[0], "k0")
    for cc in range(4):
        for ih in range(2):
            for i2 in range(2):
                fillers0.append(
                    lambda cc=cc, ih=ih, i2=i2: proj_qk_unit(
                        xnT, wk0, bk0, kT[0], cc, ih, i2, "k0"))
    wv0 = load_w_re(wv[0], "v0")
    for jc in range(TC):
        fillers0.append(
            lambda jc=jc: proj_v_unit(xnT, wv0, bvb[0], vA[0], jc, "v0"))
    attention(0, fillers0, pump=1)

    # ============ attention ob=1 with fillers ============
    wo0 = p_w.tile([128, 4, DIM], BF16, tag="wo", name="wo_0")
    nc.sync.dma_start(out=wo0, in_=wo[0].rearrange("(hd p) c -> p hd c", p=128))
    fillers1 = [lambda t=t: outproj_unit(aT, wo0, 0, t) for t in range(TC)]
    attention(1, fillers1, pump=1)

    wo1 = p_w.tile([128, 4, DIM], BF16, tag="wo", name="wo_1")
    nc.sync.dma_start(out=wo1, in_=wo[1].rearrange("(hd p) c -> p hd c", p=128))
    for t in range(TC):
        outproj_unit(aT, wo1, 1, t)


_NC = None


def _get_nc():
    global _NC
    if _NC is None:
        _NC = build_program()
    return _NC


def _make_in_maps(x1, x2, ln1_g, ln1_b, ln2_g, ln2_b,
                  w_qkv1, w_qkv2, w_out1, w_out2):
    bf16 = ml_dtypes.bfloat16
    f32 = np.float32
    branches = ((w_qkv1, ln1_g, ln1_b, w_out1), (w_qkv2, ln2_g, ln2_b, w_out2))
    # per head-group g: fold LN affine + softmax scale into weights
    per_g = []
    for g in range(2):
        cols = slice(g * QKCOLS, (g + 1) * QKCOLS)
        m = {}
        for br, (w_qkv, g_ln, b_ln, w_out) in enumerate(branches):
            wq_s = w_qkv[:, 0:DIM][:, cols]
            wk_s = w_qkv[:, DIM:2 * DIM][:, cols]
            wv_s = w_qkv[:, 2 * DIM:3 * DIM][:, cols]
            m[f"wq{br}"] = np.ascontiguousarray(
                (wq_s * g_ln[:, None] * SCALE)).astype(bf16)
            m[f"wk{br}"] = np.ascontiguousarray(wk_s * g_ln[:, None]).astype(bf16)
            m[f"wv{br}"] = np.ascontiguousarray(wv_s * g_ln[:, None]).astype(bf16)
            # q/k biases as [128, 4] per-partition columns (col cc, part p
            # holds bias for q-col cc*128+p)
            m[f"bq{br}"] = np.ascontiguousarray(
                ((b_ln @ wq_s) * SCALE).reshape(4, 128).T).astype(f32)
            m[f"bk{br}"] = np.ascontiguousarray(
                (b_ln @ wk_s).reshape(4, 128).T).astype(f32)
            m[f"bv{br}"] = (b_ln @ wv_s)[None, :].astype(bf16)
            m[f"wo{br}"] = np.ascontiguousarray(w_out[cols, :]).astype(bf16)
        per_g.append(m)
    in_maps = []
    for b in range(B):
        for g in range(2):
            m = dict(per_g[g])
            m["x0"] = np.ascontiguousarray(x1[b], dtype=f32)
            m["x1"] = np.ascontiguousarray(x2[b], dtype=f32)
            in_maps.append(m)
    return in_maps


def run(inputs, trace=False):
    """inputs: dict as from setup_inputs(). Returns ((out1, out2), exec_time_ns)."""
    from concourse.bass_utils import run_bass_kernel_spmd

    f32 = np.float32
    ins = {k: np.asarray(v) for k, v in inputs.items()}
    nc = _get_nc()
    in_maps = _make_in_maps(
        ins["x1"].astype(f32), ins["x2"].astype(f32),
        ins["ln1_g"].astype(f32), ins["ln1_b"].astype(f32),
        ins["ln2_g"].astype(f32), ins["ln2_b"].astype(f32),
        ins["w_qkv1"].astype(f32), ins["w_qkv2"].astype(f32),
        ins["w_out1"].astype(f32), ins["w_out2"].astype(f32))
    res = run_bass_kernel_spmd(nc, in_maps, core_ids=list(range(8)), trace=trace)
    r = res.results
    out1 = np.zeros((B, N, DIM), f32)
    out2 = np.zeros((B, N, DIM), f32)
    for b in range(B):
        out1[b] = r[2 * b]["o0"] + r[2 * b + 1]["o0"] + ins["b_out1"].astype(f32)
        out2[b] = r[2 * b]["o1"] + r[2 * b + 1]["o1"] + ins["b_out2"].astype(f32)
    return (out1, out2), res.exec_time_ns


def kernel(**inputs):
    (out1, out2), _ = run(inputs, trace=False)
    return out1, out2


# revision 12
# speedup vs baseline: 1.1638x; 1.0658x over previous
"""Trainium2 Bass kernel for a two-branch cross-attention block.

Math (per branch pair):
    x1n = LN(x1); x2n = LN(x2)
    q1,k1,v1 = split(x1n @ w_qkv1); q2,k2,v2 = split(x2n @ w_qkv2)
    out1 = softmax(q1 k2^T * s) v2 @ w_out1 + b_out1
    out2 = softmax(q2 k1^T * s) v1 @ w_out2 + b_out2

Sharding: 8 cores = 4 batches x 2 head-groups (8 heads each). Each core
handles both branches for its (batch, head-group); the out-projection
contracts over heads, so each core produces a partial [2048, 1024] per
branch and the host sums the two head-group partials + bias.

LN affine (g, b) is folded into the QKV weights on the host; the softmax
scale is folded into the q-side weights. q/k biases are added on the
PSUM->SBUF copy (per-partition scalar), the v bias via a broadcast
tensor-tensor add, so no K=1 bias matmuls reach the PE.

v2 layout (vs the 1.10 ms baseline):
  - AV computed operand-swapped: out[i, d] = es_chunk^T(stationary)
    @ v_aug(moving, 65 cols incl. a ones column for Z). Streams 65
    columns instead of 512 -> AV PE time halved.
  - Attention normalization: recipZ via DVE reciprocal + per-i-chunk
    fused scale on the PSUM->SBUF copy (no DRAM round trip).
  - Single LN per branch feeding all three projections; the branch-0
    LN+transpose is recomputed once as PE filler during attention.
  - Projections / out-projection interleaved as fillers between
    attention heads so the PE keeps running during the ACT-bound
    exp stretches.
"""

import sys
from contextlib import ExitStack

import numpy as np
import ml_dtypes

sys.path.insert(0, "/opt/trn_rl_repo")
sys.path.insert(0, "/opt/trn_rl_repo/concourse")

import concourse.bass as bass
import concourse.tile as tile
from concourse import bacc, mybir
from concourse.bass import ds, ts
from concourse.masks import make_identity

F32 = mybir.dt.float32
BF16 = mybir.dt.bfloat16
AF = mybir.ActivationFunctionType
ALU = mybir.AluOpType

B, N, DIM = 4, 2048, 1024
HEADS, DH = 16, 64
SCALE = DH ** -0.5
HPC = 8          # heads per core
QKCOLS = HPC * DH  # 512 qkv columns per core per tensor
TC = N // 128    # 16 token chunks
KC = DIM // 128  # 8 feature chunks
EPS = 1e-5
# Schraudolph exp: bf16 bit-pattern = trunc(x * 128/ln2 + (127*128 - c + 0.5))
# c=7.42 tuned offline; ~1% rel err on the affected softmax slices.
SCH_A = 184.6649652
SCH_B = 16249.08
# GPSIMD cannot read PSUM, so all Schraudolph exps run on DVE.
DVE_JC = (2, 5, 8, 11, 14)
POOL_JC = ()


def build_program():
    nc = bacc.Bacc(
        "TRN2",
        target_bir_lowering=False,
        debug=False,
        enable_asserts=True,
        num_devices=8,
    )
    xs, wq, wk, wv, bq, bk, bv, wo, outs = [], [], [], [], [], [], [], [], []
    for br in range(2):
        xs.append(nc.dram_tensor(f"x{br}", [N, DIM], F32, kind="ExternalInput").ap())
        wq.append(nc.dram_tensor(f"wq{br}", [DIM, QKCOLS], BF16, kind="ExternalInput").ap())
        wk.append(nc.dram_tensor(f"wk{br}", [DIM, QKCOLS], BF16, kind="ExternalInput").ap())
        wv.append(nc.dram_tensor(f"wv{br}", [DIM, QKCOLS], BF16, kind="ExternalInput").ap())
        bq.append(nc.dram_tensor(f"bq{br}", [128, 4], F32, kind="ExternalInput").ap())
        bk.append(nc.dram_tensor(f"bk{br}", [128, 4], F32, kind="ExternalInput").ap())
        bv.append(nc.dram_tensor(f"bv{br}", [1, QKCOLS], BF16, kind="ExternalInput").ap())
        wo.append(nc.dram_tensor(f"wo{br}", [QKCOLS, DIM], BF16, kind="ExternalInput").ap())
        outs.append(nc.dram_tensor(f"o{br}", [N, DIM], F32, kind="ExternalOutput").ap())

    with tile.TileContext(nc) as tc:
        with ExitStack() as ctx:
            _body(ctx, tc, xs, wq, wk, wv, bq, bk, bv, wo, outs)
    nc.finalize()
    return nc


def _body(ctx, tc, xs, wq, wk, wv, bq, bk, bv, wo, outs):
    nc = tc.nc
    p_const = ctx.enter_context(tc.tile_pool(name="const", bufs=1))
    p_x = ctx.enter_context(tc.tile_pool(name="x", bufs=2))
    p_stat = ctx.enter_context(tc.tile_pool(name="stat", bufs=1))
    p_z = ctx.enter_context(tc.tile_pool(name="z", bufs=2))
    p_w = ctx.enter_context(tc.tile_pool(name="w", bufs=1))
    p_big = ctx.enter_context(tc.tile_pool(name="big", bufs=1))
    p_es = ctx.enter_context(tc.tile_pool(name="es", bufs=2))
    p_outst = ctx.enter_context(tc.tile_pool(name="outst", bufs=2))
    # PSUM budget (8 banks): S double-buffer 2x2, AV accumulator 2 (i in
    # halves per head), fillers/transposes/projections 2.
    ps_s = ctx.enter_context(tc.tile_pool(name="ps_s", bufs=2, space="PSUM"))
    ps_mm = ctx.enter_context(tc.tile_pool(name="ps_mm", bufs=1, space="PSUM"))
    ps_av = ctx.enter_context(tc.tile_pool(name="ps_av", bufs=1, space="PSUM"))

    ident = p_const.tile([128, 128], BF16, tag="ident", name="ident")
    make_identity(nc, ident)

    # per-branch LN stats kept alive for the branch-0 recompute
    stats = [None, None]
    rstd = [None, None]
    nmr = [None, None]

    def ln_stats(br):
        """Pass 1 over x[br]: bn stats, rstd, -mu*rstd (all [128, TC])."""
        st_all = p_stat.tile([128, TC, 2], F32, tag=f"stats{br}",
                             name=f"stats_{br}")
        rs = p_stat.tile([128, TC], F32, tag=f"rstd{br}", name=f"rstd_{br}")
        nm = p_stat.tile([128, TC], F32, tag=f"nmr{br}", name=f"nmr_{br}")
        epst = p_stat.tile([128, 1], F32, tag=f"eps{br}", name=f"epst_{br}")
        nc.vector.memset(epst, EPS)
        for t in range(TC):
            xt = p_x.tile([128, DIM], F32, tag="xt", name=f"xs{br}_{t}")
            nc.sync.dma_start(out=xt, in_=xs[br][ts(t, 128), :])
            st = p_stat.tile([128, 2, 6], F32, tag="st", name=f"st{br}_{t}")
            for sg in range(2):
                nc.vector.bn_stats(out=st[:, sg, :], in_=xt[:, ts(sg, 512)])
            nc.vector.bn_aggr(out=st_all[:, t, :], in_=st)
        # rstd = exp(-0.5 * ln(var + eps)), batched over all 16 chunks
        nc.scalar.activation(out=rs, in_=st_all[:, :, 1], func=AF.Ln,
                             bias=epst, scale=1.0)
        nc.scalar.activation(out=rs, in_=rs, func=AF.Exp, scale=-0.5)
        # nmr = -mu * rstd (bias for the ACT/DVE normalize apply)
        nc.vector.tensor_tensor(out=nm, in0=st_all[:, :, 0], in1=rs,
                                op=ALU.mult)
        nc.vector.tensor_scalar(out=nm, in0=nm, scalar1=-1.0, scalar2=None,
                                op0=ALU.mult)
        stats[br], rstd[br], nmr[br] = st_all, rs, nm

    def ln_apply_unit(br, t, xnT, seg, on_act):
        """Pass 2, one token chunk: z = x*rstd - mu*rstd (bf16), transpose
        into xnT[:, :, t*128:(t+1)*128]."""
        xt = p_x.tile([128, DIM], F32, tag="xt", name=f"xa{seg}_{t}")
        nc.sync.dma_start(out=xt, in_=xs[br][ts(t, 128), :])
        zt = p_z.tile([128, DIM], BF16, tag="zt", name=f"zt{seg}_{t}")
        if on_act:
            nc.scalar.activation(out=zt, in_=xt, func=AF.Identity,
                                 bias=nmr[br][:, t:t + 1],
                                 scale=rstd[br][:, t:t + 1])
        else:
            nc.gpsimd.tensor_scalar(out=zt, in0=xt,
                                    scalar1=stats[br][:, t, 0:1],
                                    scalar2=rstd[br][:, t:t + 1],
                                    op0=ALU.subtract, op1=ALU.mult)
        ptr = ps_mm.tile([128, KC, 128], BF16, tag="mm", name=f"ptr{seg}_{t}")
        for fc in range(KC):
            nc.tensor.transpose(out=ptr[:, fc, :], in_=zt[:, ts(fc, 128)],
                                identity=ident)
        nc.vector.tensor_copy(out=xnT[:, :, ts(t, 128)], in_=ptr)

    def load_w_re(wt_d, lbl):
        wsb = p_w.tile([128, KC, QKCOLS], BF16, tag="w", bufs=1,
                       name=f"w_{lbl}")
        nc.sync.dma_start(out=wsb,
                          in_=wt_d.rearrange("(kc p) c -> p kc c", p=128))
        return wsb

    def load_bias(bias_d, lbl):
        bsb = p_stat.tile([128, 4], F32, tag=f"b_{lbl}", name=f"b_{lbl}")
        nc.sync.dma_start(out=bsb, in_=bias_d)
        return bsb

    def proj_qk_unit(xnT, wsb, bsb, dstT, cc, ih, i2, lbl):
        """One [128 cols x 512 tokens] chunk of a q/k projection,
        transposed output layout, bias fused into the copy."""
        ps = ps_mm.tile([128, 512], F32, tag="mm",
                        name=f"psB_{lbl}_{cc}_{ih}_{i2}")
        for k in range(KC):
            nc.tensor.matmul(
                out=ps, lhsT=wsb[:, k, ts(cc, 128)],
                rhs=xnT[:, k, ds(ih * 1024 + i2 * 512, 512)],
                start=(k == 0), stop=(k == KC - 1))
        nc.vector.tensor_scalar(out=dstT[:, cc, ds(ih * 1024 + i2 * 512, 512)],
                                in0=ps, scalar1=bsb[:, cc:cc + 1],
                                scalar2=None, op0=ALU.add)

    def proj_v_unit(xnT, wsb, bvb, vA, jc, lbl):
        """v for one j-chunk, natural layout [j, h, d], bias row added."""
        ps = ps_mm.tile([128, 1024], F32, tag="mm", name=f"psC_{lbl}_{jc}")
        for k in range(KC):
            nc.tensor.matmul(out=ps[:, 0:512], lhsT=xnT[:, k, ts(jc, 128)],
                             rhs=wsb[:, k, :], start=(k == 0),
                             stop=(k == KC - 1))
        nc.vector.tensor_tensor(
            out=vA[:, jc, :, 0:DH],
            in0=ps[:, 0:512].rearrange("p (h d) -> p h d", d=DH),
            in1=bvb.rearrange("p (h d) -> p h d", d=DH), op=ALU.add)

    def outproj_unit(aT, wosb, ob, t, pool=None, copy_act=False):
        pool = pool or ps_mm
        tag = "s" if pool is ps_s else "mm"
        ps = pool.tile([128, 1024], F32, tag=tag, name=f"psE_{ob}_{t}")
        for hd in range(4):
            for cb in range(2):
                nc.tensor.matmul(out=ps[:, ts(cb, 512)],
                                 lhsT=aT[:, hd, ts(t, 128)],
                                 rhs=wosb[:, hd, ts(cb, 512)],
                                 start=(hd == 0), stop=(hd == 3))
        ot = p_outst.tile([128, DIM], F32, tag="ot", name=f"ot_{ob}_{t}")
        if copy_act:
            nc.scalar.activation(out=ot, in_=ps, func=AF.Copy)
        else:
            nc.vector.tensor_copy(out=ot, in_=ps)
        nc.sync.dma_start(out=outs[ob][ts(t, 128), :], in_=ot)

    # ---- persistent big tiles ----
    xnT = p_big.tile([128, KC, N], BF16, tag="xnT", name="xnT")
    qT = [p_big.tile([128, 4, N], BF16, tag=f"qT{br}", name=f"qT_{br}")
          for br in range(2)]
    kT = [p_big.tile([128, 4, N], BF16, tag=f"kT{br}", name=f"kT_{br}")
          for br in range(2)]
    vA = [p_big.tile([128, TC, HPC, DH + 1], BF16, tag=f"vA{br}",
                     name=f"vA_{br}") for br in range(2)]
    aT = p_big.tile([128, 4, N], BF16, tag="aT", name="aT")
    attn = p_big.tile([128, TC, HPC, DH], BF16, tag="attn", name="attn")
    rz = p_big.tile([128, TC, HPC], F32, tag="rz", name="rz")
    for br in range(2):
        nc.vector.memset(vA[br][:, :, :, DH:DH + 1], 1.0)

    bvb = [None, None]
    for br in range(2):
        bvb[br] = p_const.tile([128, QKCOLS], BF16, tag=f"bvb{br}",
                               name=f"bvb_{br}")
        nc.sync.dma_start(out=bvb[br], in_=bv[br].partition_broadcast(128))

    def attention(ob, fillers, pump):
        """ob: 0 -> q from branch 0, k/v from branch 1; 1 -> reverse."""
        sb = 1 - ob
        my_q, my_k, my_v = qT[ob], kT[sb], vA[sb]
        fi = [0]  # filler cursor

        def do_fill(n):
            for _ in range(n):
                if fi[0] < len(fillers):
                    fillers[fi[0]]()
                    fi[0] += 1

        pend = [None]  # deferred AV round: (av, es, jc, h)

        def av_flush():
            if pend[0] is None:
                return
            av, es, jc, h = pend[0]
            pend[0] = None
            # PSUM accumulation groups are per 2KB bank: av bank b holds
            # i-chunks 4b..4b+3, so start/stop only on the bank's first/last
            # write of the whole jc loop.
            for ic in range(8):
                nc.tensor.matmul(out=av[:, ic, 0:DH + 1],
                                 lhsT=es[:, ts(ic, 128)],
                                 rhs=my_v[:, jc, h, :],
                                 start=(jc == 0 and ic % 4 == 0),
                                 stop=(jc == TC - 1 and ic % 4 == 3))

        jcount = [0]
        for h in range(HPC):
            pt, po = h // 2, (h % 2) * 64
            for ihalf in range(2):
                av = ps_av.tile([128, 8, 128], F32, tag="av",
                                name=f"av_{ob}_{h}_{ihalf}")
                for jc in range(TC):
                    es = p_es.tile([128, 1024], BF16, tag="es",
                                   name=f"es_{ob}_{h}_{ihalf}_{jc}")
                    ps = ps_s.tile([128, 1024], F32, tag="s",
                                   name=f"psS_{ob}_{h}_{ihalf}_{jc}")
                    for i2 in range(2):
                        nc.tensor.matmul(
                            out=ps[:, ts(i2, 512)],
                            lhsT=my_k[po:po + 64, pt, ts(jc, 128)],
                            rhs=my_q[po:po + 64, pt,
                                     ds(ihalf * 1024 + i2 * 512, 512)],
                            start=True, stop=True)
                    if jc in DVE_JC:
                        nc.vector.tensor_scalar(
                            out=es[:, :].bitcast(mybir.dt.int16), in0=ps,
                            scalar1=SCH_A, scalar2=SCH_B,
                            op0=ALU.mult, op1=ALU.add)
                    elif jc in POOL_JC:
                        nc.gpsimd.tensor_scalar(
                            out=es[:, :].bitcast(mybir.dt.int16), in0=ps,
                            scalar1=SCH_A, scalar2=SCH_B,
                            op0=ALU.mult, op1=ALU.add)
                    else:
                        nc.scalar.activation(out=es, in_=ps, func=AF.Exp)
                    # software pipeline: previous AV lands after this S so
                    # the PE isn't parked behind the exp it needs
                    av_flush()
                    pend[0] = (av, es, jc, h)
                    jcount[0] += 1
                    if jcount[0] % 3 == 0:
                        do_fill(1)
                av_flush()
                # epilogue: recipZ, fused normalize into attn staging
                ib = ihalf * 8
                nc.vector.reciprocal(out=rz[:, ds(ib, 8), h:h + 1],
                                     in_=av[:, :, DH:DH + 1])
                for ic in range(8):
                    nc.vector.tensor_scalar(
                        out=attn[:, ib + ic, h, :], in0=av[:, ic, 0:DH],
                        scalar1=rz[:, ib + ic, h:h + 1], scalar2=None,
                        op0=ALU.mult)
        av_flush()
        # drain remaining fillers
        do_fill(len(fillers))
        # transpose attn [i, (h d)] -> aT [(h d), i]
        for ic in range(TC):
            tp = ps_mm.tile([128, 4, 128], BF16, tag="mm", name=f"tp{ob}_{ic}")
            for b4 in range(4):
                nc.tensor.transpose(out=tp[:, b4, :],
                                    in_=attn[:, ic, ds(2 * b4, 2), :],
                                    identity=ident)
            nc.vector.tensor_copy(out=aT[:, :, ts(ic, 128)], in_=tp)

    # ================= prologue =================
    ln_stats(0)
    for t in range(TC):
        ln_apply_unit(0, t, xnT, "p0", on_act=True)
    wq0 = load_w_re(wq[0], "q0")
    bq0 = load_bias(bq[0], "q0")
    ln_stats(1)  # DVE stats overlap the q1 projection below
    for cc in range(4):
        for ih in range(2):
            for i2 in range(2):
                proj_qk_unit(xnT, wq0, bq0, qT[0], cc, ih, i2, "q0")
    for t in range(TC):
        ln_apply_unit(1, t, xnT, "p1", on_act=True)
    wk1 = load_w_re(wk[1], "k1")
    bk1 = load_bias(bk[1], "k1")
    for cc in range(4):
        for ih in range(2):
            for i2 in range(2):
                proj_qk_unit(xnT, wk1, bk1, kT[1], cc, ih, i2, "k1")
    wv1 = load_w_re(wv[1], "v1")
    for jc in range(TC):
        proj_v_unit(xnT, wv1, bvb[1], vA[1], jc, "v1")
    wq1 = load_w_re(wq[1], "q1")
    bq1 = load_bias(bq[1], "q1")
    for cc in range(4):
        for ih in range(2):
            for i2 in range(2):
                proj_qk_unit(xnT, wq1, bq1, qT[1], cc, ih, i2, "q1")

    # ============ attention ob=0 with fillers ============
    # fillers: recompute branch-0 LN/transpose (DVE z-apply), then k1/v1
    fillers0 = []
    for t in range(TC):
        fillers0.append(
            lambda t=t: ln_apply_unit(0, t, xnT, "r0", on_act=False))
    wk0 = load_w_re(wk[0], "k0")
    bk0 = load_bias(bk[0], "k0")
    for cc in range(4):
        for ih in range(2):
            for i2 in range(2):
                fillers0.append(
                    lambda cc=cc, ih=ih, i2=i2: proj_qk_unit(
                        xnT, wk0, bk0, kT[0], cc, ih, i2, "k0"))
    wv0 = load_w_re(wv[0], "v0")
    for jc in range(TC):
        fillers0.append(
            lambda jc=jc: proj_v_unit(xnT, wv0, bvb[0], vA[0], jc, "v0"))
    attention(0, fillers0, pump=1)

    # ============ attention ob=1 with fillers ============
    wo0 = p_w.tile([128, 4, DIM], BF16, tag="wo", name="wo_0")
    nc.sync.dma_start(out=wo0, in_=wo[0].rearrange("(hd p) c -> p hd c", p=128))
    fillers1 = [lambda t=t: outproj_unit(aT, wo0, 0, t) for t in range(TC)]
    attention(1, fillers1, pump=1)

    wo1 = p_w.tile([128, 4, DIM], BF16, tag="wo", name="wo_1")
    nc.sync.dma_start(out=wo1, in_=wo[1].rearrange("(hd p) c -> p hd c", p=128))
    for t in range(TC):
        outproj_unit(aT, wo1, 1, t, pool=ps_s, copy_act=(t % 2 == 1))


_NC = None


def _get_nc():
    global _NC
    if _NC is None:
        _NC = build_program()
    return _NC


def _make_in_maps(x1, x2, ln1_g, ln1_b, ln2_g, ln2_b,
                  w_qkv1, w_qkv2, w_out1, w_out2):
    bf16 = ml_dtypes.bfloat16
    f32 = np.float32
    branches = ((w_qkv1, ln1_g, ln1_b, w_out1), (w_qkv2, ln2_g, ln2_b, w_out2))
    # per head-group g: fold LN affine + softmax scale into weights
    per_g = []
    for g in range(2):
        cols = slice(g * QKCOLS, (g + 1) * QKCOLS)
        m = {}
        for br, (w_qkv, g_ln, b_ln, w_out) in enumerate(branches):
            wq_s = w_qkv[:, 0:DIM][:, cols]
            wk_s = w_qkv[:, DIM:2 * DIM][:, cols]
            wv_s = w_qkv[:, 2 * DIM:3 * DIM][:, cols]
            m[f"wq{br}"] = np.ascontiguousarray(
                (wq_s * g_ln[:, None] * SCALE)).astype(bf16)
            m[f"wk{br}"] = np.ascontiguousarray(wk_s * g_ln[:, None]).astype(bf16)
            m[f"wv{br}"] = np.ascontiguousarray(wv_s * g_ln[:, None]).astype(bf16)
            # q/k biases as [128, 4] per-partition columns (col cc, part p
            # holds bias for q-col cc*128+p)
            m[f"bq{br}"] = np.ascontiguousarray(
                ((b_ln @ wq_s) * SCALE).reshape(4, 128).T).astype(f32)
            m[f"bk{br}"] = np.ascontiguousarray(
                (b_ln @ wk_s).reshape(4, 128).T).astype(f32)
            m[f"bv{br}"] = (b_ln @ wv_s)[None, :].astype(bf16)
            m[f"wo{br}"] = np.ascontiguousarray(w_out[cols, :]).astype(bf16)
        per_g.append(m)
    in_maps = []
    for b in range(B):
        for g in range(2):
            m = dict(per_g[g])
            m["x0"] = np.ascontiguousarray(x1[b], dtype=f32)
            m["x1"] = np.ascontiguousarray(x2[b], dtype=f32)
            in_maps.append(m)
    return in_maps


def run(inputs, trace=False):
    """inputs: dict as from setup_inputs(). Returns ((out1, out2), exec_time_ns)."""
    from concourse.bass_utils import run_bass_kernel_spmd

    f32 = np.float32
    ins = {k: np.asarray(v) for k, v in inputs.items()}
    nc = _get_nc()
    in_maps = _make_in_maps(
        ins["x1"].astype(f32), ins["x2"].astype(f32),
        ins["ln1_g"].astype(f32), ins["ln1_b"].astype(f32),
        ins["ln2_g"].astype(f32), ins["ln2_b"].astype(f32),
        ins["w_qkv1"].astype(f32), ins["w_qkv2"].astype(f32),
        ins["w_out1"].astype(f32), ins["w_out2"].astype(f32))
    res = run_bass_kernel_spmd(nc, in_maps, core_ids=list(range(8)), trace=trace)
    r = res.results
    out1 = np.zeros((B, N, DIM), f32)
    out2 = np.zeros((B, N, DIM), f32)
    for b in range(B):
        out1[b] = r[2 * b]["o0"] + r[2 * b + 1]["o0"] + ins["b_out1"].astype(f32)
        out2[b] = r[2 * b]["o1"] + r[2 * b + 1]["o1"] + ins["b_out2"].astype(f32)
    return (out1, out2), res.exec_time_ns


def kernel(**inputs):
    (out1, out2), _ = run(inputs, trace=False)
    return out1, out2
